# revision 25
# baseline (speedup 1.0000x reference)
"""FGN (fuzzy Gaussian neuron) layer on 8 TRN2 NeuronCores.

Math (reference, fp32):
    l = x @ W.T + b                                  [B, OUT]
    g = exp(-sum_i ((x_bi - c_zi) * ic_zi)^2)        [B, OUT]
    returns (l * g, g)

The Gaussian distance expands into matmuls:
    d[b,z] = sum_i x^2 * s2 - 2 * sum_i x * (c*s2) + sum_i c^2*s2
with s2 = min(ic, 1e8)^2.  So everything is tensor-engine work:
    e = (-s2) @ x2 + (2*c*s2) @ x      (PSUM accumulate)
    g = exp(e - sum_i c^2*s2)          (ACT exp with per-partition bias)
    res = (l + b) * g                  (one fused DVE scalar_tensor_tensor)

Precision/speed strategy (fp32 matmuls are 4 cyc/row on TRN2 PE; bf16 is 1):
  * linear part: bf16 hi/lo split (x = xh + xl, w = wh + wl) ->
    l = xh*wh + xh*wl + xl*wh, three 1-cyc/row bf16 passes, error ~2^-18.
  * Gaussian exponent: x^2 rounded once from fp32 to bf16 on the host;
    the tightly-clustered s2 panel is hi/lo split (its bf16 quantization
    error is otherwise systematic).  g error ~2.4e-5 absolute.

DMA strategy (each dma_start costs ~650ns of descriptor-gen time):
  * x panels (xh, xl, x2) are separate bf16 tensors, loaded in k-chunks
    ordered by first use so matmuls start before all of x has landed;
  * all five weight panels packed into ONE [IN, 5*ZS] bf16 tensor;
  * both biases packed into ONE [ZS, 2] fp32 tensor (tiny -> SWDGE lane);
  * g and res write into ONE packed [ZS, NB, 2, NF] fp32 output per core;
  * loads/stores alternate across the two HWDGE-capable engines.

A few dummy matmuls + one dummy exp run during the DMA prologue to trip
the PE HAM clock ramp (cold PE runs at half clock for its first ~3.4us)
and the ACT exp-table load (~2.7us) while the wires are busy anyway.

Sharding: OUT (=1024) split across 8 cores, 128 rows each (tensor parallel
over neurons).  x is replicated, transposed on host so both matmul operands
are K-major.  Per-core output is z-major; host concats + transposes.
"""

import os
import numpy as np
import ml_dtypes

import concourse.bacc as bacc
import concourse.mybir as mybir
import concourse.tile as tile
from concourse.bass_utils import run_bass_kernel_spmd

B, IN, OUT = 1024, 256, 1024
NCORES = 8
ZS = OUT // NCORES          # out-rows per core
KP = 128                    # contraction chunk (partition dim)
KC = IN // KP               # number of contraction chunks
NF = 512                    # moving free-dim per matmul (one fp32 PSUM bank)
NB = B // NF                # number of moving chunks
F32 = mybir.dt.float32
BF16 = mybir.dt.bfloat16

EPS = 1e-08

N_WARMUP_MM = int(os.environ.get("FGN_WARMUP_MM", "4"))
# Bench mode: unroll the whole body N times inside one NEFF so per-iteration
# hardware time can be measured as (wall(N) - wall(1)) / (N - 1).
ITERS = int(os.environ.get("FGN_ITERS", "1"))

_CACHE = {}


def _build_nc():
    nc = bacc.Bacc("TRN2", target_bir_lowering=False, debug=False,
                   num_devices=NCORES)
    xh = nc.dram_tensor("xh", [IN, B], BF16, kind="ExternalInput")
    xl = nc.dram_tensor("xl", [IN, B], BF16, kind="ExternalInput")
    x2 = nc.dram_tensor("x2", [IN, B], BF16, kind="ExternalInput")
    # packed weight panels along the free axis: [wh | wl | at | sh | sl]
    w5 = nc.dram_tensor("w5", [IN, 5 * ZS], BF16, kind="ExternalInput")
    bb = nc.dram_tensor("bb", [ZS, 2], F32, kind="ExternalInput")
    gres = nc.dram_tensor("gres", [ZS, NB, 2, NF], F32, kind="ExternalOutput")

    AF = mybir.ActivationFunctionType
    ALU = mybir.AluOpType

    with tile.TileContext(nc) as tc:
        with (
            tc.tile_pool(name="const", bufs=2) as cpool,
            tc.tile_pool(name="work", bufs=2) as wpool,
            tc.tile_pool(name="psum", bufs=2, space="PSUM") as ppool,
        ):
            # --- warmup: PE clock ramp + ACT exp-table load
            wu = cpool.tile([KP, NF], BF16, name="wu", tag="wu", bufs=1)
            nc.vector.memset(wu[:], 0.0)
            wu_act = cpool.tile([KP, 1], F32, name="wu_act", tag="wua", bufs=1)
            nc.scalar.activation(wu_act[:], wu[:, 0:1], AF.Exp)
            wu_ps = ppool.tile([KP, NF], F32, name="wu_ps", tag="wu", bufs=1)
            for i in range(N_WARMUP_MM):
                nc.tensor.matmul(wu_ps[:], wu[:, 0:KP], wu[:],
                                 start=True, stop=True)

            for it in range(ITERS):
                _emit_iter(nc, tc, cpool, wpool, ppool, it,
                           xh, xl, x2, w5, bb, gres, AF, ALU)
    nc.compile()
    return nc


def _emit_iter(nc, tc, cpool, wpool, ppool, it, xh, xl, x2, w5, bb, gres,
               AF, ALU):
    if True:
        if True:
            xht = cpool.tile([KP, KC, B], BF16, name=f"xht{it}", tag="xht",
                             bufs=2)
            xlt = cpool.tile([KP, KC, B], BF16, name=f"xlt{it}", tag="xlt",
                             bufs=2)
            x2t = cpool.tile([KP, KC, B], BF16, name=f"x2t{it}", tag="x2t",
                             bufs=2)
            w5t = cpool.tile([KP, KC, 5 * ZS], BF16, name=f"w5t{it}",
                             tag="w5t", bufs=2)
            bbt = cpool.tile([ZS, 2], F32, name=f"bbt{it}", tag="bbt", bufs=2)

            # --- loads, in first-use order; weights + biases ride the
            # otherwise-idle SWDGE (gpsimd) lane so their descriptor-gen
            # doesn't serialize behind the x loads on the HWDGE lanes.
            k0, k1 = slice(0, KP), slice(KP, 2 * KP)
            nc.gpsimd.dma_start(out=bbt[:], in_=bb[:])
            nc.scalar.dma_start(out=w5t[:, 0, :], in_=w5[k0, :])
            nc.sync.dma_start(out=xht[:, 0, :], in_=xh[k0, :])
            nc.sync.dma_start(out=x2t[:, 0, :], in_=x2[k0, :])
            nc.scalar.dma_start(out=w5t[:, 1, :], in_=w5[k1, :])
            nc.sync.dma_start(out=xht[:, 1, :], in_=xh[k1, :])
            nc.sync.dma_start(out=x2t[:, 1, :], in_=x2[k1, :])
            nc.scalar.dma_start(out=xlt[:, 0, :], in_=xl[k0, :])
            nc.scalar.dma_start(out=xlt[:, 1, :], in_=xl[k1, :])

            def wp(k, j):  # panel j of chunk k: wh=0 wl=1 at=2 sh=3 sl=4
                return w5t[:, k, j * ZS:(j + 1) * ZS]

            blt = bbt[:, 0:1]
            bet = bbt[:, 1:2]

            l_ps, e_ps, grt = [None] * NB, [None] * NB, [None] * NB
            for bc in range(NB):
                l_ps[bc] = ppool.tile([KP, NF], F32, name=f"l_ps{bc}_{it}",
                                      tag=f"l{bc}", bufs=1)
                e_ps[bc] = ppool.tile([KP, NF], F32, name=f"e_ps{bc}_{it}",
                                      tag=f"e{bc}", bufs=1)
                grt[bc] = wpool.tile([KP, 2, NF], F32, name=f"grt{bc}_{it}",
                                     tag=f"gr{bc}", bufs=1)

            # --- matmuls, k-outer so chunk-0 compute overlaps chunk-1 DMA;
            # in the last k-chunk, finish bc1 first so its epilogue overlaps
            # bc0's remaining matmuls (bc0 then gets the fine-grained tail).
            for k in range(KC):
                first, last_k = k == 0, k == KC - 1
                bcs = range(NB) if not last_k else range(NB - 1, -1, -1)
                for bc in bcs:
                    bs = slice(bc * NF, (bc + 1) * NF)
                    if not last_k:
                        nc.tensor.matmul(l_ps[bc][:], wp(k, 0), xht[:, k, bs],
                                         start=first, stop=False)
                        nc.tensor.matmul(e_ps[bc][:], wp(k, 2), xht[:, k, bs],
                                         start=first, stop=False)
                        nc.tensor.matmul(l_ps[bc][:], wp(k, 1), xht[:, k, bs],
                                         start=False, stop=False)
                        nc.tensor.matmul(e_ps[bc][:], wp(k, 3), x2t[:, k, bs],
                                         start=False, stop=False)
                        nc.tensor.matmul(e_ps[bc][:], wp(k, 4), x2t[:, k, bs],
                                         start=False, stop=False)
                        nc.tensor.matmul(l_ps[bc][:], wp(k, 0), xlt[:, k, bs],
                                         start=False, stop=False)
                    else:
                        # e-psum stops first so exp overlaps the l matmuls
                        nc.tensor.matmul(e_ps[bc][:], wp(k, 2), xht[:, k, bs],
                                         start=False, stop=False)
                        nc.tensor.matmul(e_ps[bc][:], wp(k, 3), x2t[:, k, bs],
                                         start=False, stop=False)
                        nc.tensor.matmul(e_ps[bc][:], wp(k, 4), x2t[:, k, bs],
                                         start=False, stop=True)
                        nc.tensor.matmul(l_ps[bc][:], wp(k, 0), xht[:, k, bs],
                                         start=False, stop=False)
                        nc.tensor.matmul(l_ps[bc][:], wp(k, 1), xht[:, k, bs],
                                         start=False, stop=False)
                        nc.tensor.matmul(l_ps[bc][:], wp(k, 0), xlt[:, k, bs],
                                         start=False, stop=True)

            # --- epilogues: exp + g-store, then res + store; the chunk
            # finishing last (bc0 after the k-reversal) gets a halved tail.
            for bc in range(NB - 1, -1, -1):
                nc.scalar.activation(grt[bc][:, 0, :], e_ps[bc][:], AF.Exp,
                                     bias=bet)
                eng = nc.sync if bc % 2 == 0 else nc.scalar
                eng.dma_start(out=gres[:, bc, 0, :], in_=grt[bc][:, 0, :])
            for bc in range(NB - 1, -1, -1):
                if bc == 0:
                    h = NF // 2
                    for ci in range(2):
                        cs = slice(ci * h, (ci + 1) * h)
                        nc.vector.scalar_tensor_tensor(
                            grt[bc][:, 1, cs], l_ps[bc][:, cs], blt,
                            grt[bc][:, 0, cs], op0=ALU.add, op1=ALU.mult)
                        eng = nc.scalar if ci % 2 == 0 else nc.sync
                        eng.dma_start(out=gres[:, bc, 1, cs],
                                      in_=grt[bc][:, 1, cs])
                else:
                    nc.vector.scalar_tensor_tensor(
                        grt[bc][:, 1, :], l_ps[bc][:], blt, grt[bc][:, 0, :],
                        op0=ALU.add, op1=ALU.mult)
                    eng = nc.scalar if bc % 2 == 0 else nc.sync
                    eng.dma_start(out=gres[:, bc, 1, :], in_=grt[bc][:, 1, :])


def _get_nc():
    if "nc" not in _CACHE:
        _CACHE["nc"] = _build_nc()
    return _CACHE["nc"]


def run_in_maps(in_maps):
    nc = _get_nc()
    return run_bass_kernel_spmd(nc, in_maps, list(range(NCORES)))


def _bf16_split(a):
    """a (fp32) -> (hi, lo) bf16 with hi + lo ~ a to ~17 mantissa bits."""
    hi = a.astype(ml_dtypes.bfloat16)
    lo = (a - hi.astype(np.float32)).astype(ml_dtypes.bfloat16)
    return hi, lo


def kernel(inputs, weights, biases, centers, inv_covars):
    x = np.asarray(inputs, dtype=np.float32)
    w = np.asarray(weights, dtype=np.float32)
    b = np.asarray(biases, dtype=np.float32)
    c = np.asarray(centers, dtype=np.float32)
    ic = np.asarray(inv_covars, dtype=np.float32)

    # Host-side prep (elementwise O(B*IN)/O(OUT*IN), trivial vs the
    # O(B*OUT*IN) device work): fold clamp/sign/scale, split to bf16.
    s2 = np.minimum(ic, np.float32(1.0 / EPS))
    s2 = s2 * s2                                  # scale^2 = ic^2
    a2 = np.float32(2.0) * c * s2                 # 2*c*s2
    kz = np.sum(c * c * s2, axis=1)               # [OUT]

    xT = np.ascontiguousarray(x.T)                # [IN, B]
    xh, xl = _bf16_split(xT)
    x2 = (xT * xT).astype(ml_dtypes.bfloat16)

    in_maps = []
    for ci in range(NCORES):
        sl = slice(ci * ZS, (ci + 1) * ZS)
        whs, wls = _bf16_split(np.ascontiguousarray(w[sl].T))
        shs, sls = _bf16_split(np.ascontiguousarray(-s2[sl].T))
        w5 = np.concatenate([
            whs,
            wls,
            a2[sl].T.astype(ml_dtypes.bfloat16),
            shs,
            sls,
        ], axis=1)                                # [IN, 5*ZS] bf16
        bb = np.stack([b[sl], -kz[sl]], axis=1)   # [ZS, 2] f32
        in_maps.append({
            "xh": xh,
            "xl": xl,
            "x2": x2,
            "w5": np.ascontiguousarray(w5),
            "bb": np.ascontiguousarray(bb),
        })

    nc = _get_nc()
    out = run_bass_kernel_spmd(nc, in_maps, list(range(NCORES)))
    # gres: [ZS, NB, 2, NF] -> g = [...,0,:], res = [...,1,:]
    gs, rs = [], []
    for r in out.results:
        gr = r["gres"]
        gs.append(gr[:, :, 0, :].reshape(ZS, B))
        rs.append(gr[:, :, 1, :].reshape(ZS, B))
    res = np.concatenate(rs, axis=0).T
    g = np.concatenate(gs, axis=0).T
    return (np.ascontiguousarray(res), np.ascontiguousarray(g))
